# revision 3
# baseline (speedup 1.0000x reference)
"""KNN-Attention Trainium2 kernel, v3.

Sharding: 8 cores = 4 batches x 2 head-groups (8 heads each).
Each core computes a partial output [T, E] = combined_slice @ W_proj_slice;
host sums the two partials per batch.

v3 changes vs baseline:
  - memory-attention score path stays exact fp32 (softmax scale E*sqrt(H)=4096
    amplifies q/k errors into slot flips; even fp16 scores fail the 2e-2
    gate), but the q projection uses float32r matmuls, which the PE runs at
    1 cycle/row for 512-wide tiles (plain fp32 costs 4).
  - everything else runs in fp16 (instead of bf16): same PE/DVE cost, 8x less
    rounding error.
  - causal diagonal handled with 128-wide tiles (off-diagonal stays 512-wide):
    ~37% less exp/mask/matmul work in the diagonal region.
  - (1-g) folded into W_v columns, g into mem_v on host.
  - software pipeline: attention + c_proj of chunk c-1 are emitted interleaved
    with the qkv projections of chunk c, so the PE fills the gaps of the
    ACT-throttled exp stream and vice versa.
  - elementwise work spread across DVE / Pool(GPSIMD) / ACT.
"""

import ml_dtypes
import numpy as np

import concourse.bass as bass
import concourse.mybir as mybir
import concourse.tile as tile
from concourse import bacc
from concourse.bass_utils import run_bass_kernel_spmd

B, T, E, H, KSLOT = 4, 2048, 1024, 16, 3
D = E // H          # 64
HPC = 8             # heads per core
EC = HPC * D        # 512 cols per core
NCORES = 8
TC = 512            # t-chunk
NCHUNK = T // TC    # 4

f32 = mybir.dt.float32
f32r = mybir.dt.float32r
f16 = mybir.dt.float16
f8 = mybir.dt.float8e4
WS = 16.0           # host-side scale on W_attn so fp8 stays out of subnormals

_CACHE = {}


def _build_nc():
    nc = bacc.Bacc("TRN2", target_bir_lowering=False, debug=False)

    # ---- DRAM I/O ----
    xT = nc.dram_tensor("xT", [E, T], f8, kind="ExternalInput").ap()
    xTf = nc.dram_tensor("xTf", [E, T], f32r, kind="ExternalInput").ap()
    wqkv = nc.dram_tensor("wqkv", [E, 3 * EC], f8, kind="ExternalInput").ap()
    wq32 = nc.dram_tensor("wq32", [E, EC], f32r, kind="ExternalInput").ap()
    wp = nc.dram_tensor("wp", [EC, E], f16, kind="ExternalInput").ap()
    mk = nc.dram_tensor("mk", [T, KSLOT * EC], f32, kind="ExternalInput").ap()
    mvg = nc.dram_tensor("mvg", [T, KSLOT * EC], f16, kind="ExternalInput").ap()
    mask = nc.dram_tensor("mask", [128, 128], f16, kind="ExternalInput").ap()
    ident = nc.dram_tensor("ident", [128, 128], f16, kind="ExternalInput").ap()
    out = nc.dram_tensor("out", [T, E], f32, kind="ExternalOutput").ap()

    # partition-tiled DRAM views
    xT_r = xT.rearrange("(ko p) t -> p ko t", p=128)        # [128, 8, T]
    xTf_r = xTf.rearrange("(ko p) t -> p ko t", p=128)      # [128, 8, T]
    wqkv_r = wqkv.rearrange("(ko p) n -> p ko n", p=128)    # [128, 8, 1536]
    wq32_r = wq32.rearrange("(ko p) n -> p ko n", p=128)    # [128, 8, 512]
    wp_r = wp.rearrange("(ko p) n -> p ko n", p=128)        # [128, 4, 1024]

    with tile.TileContext(nc) as tc:
        with (
            tc.tile_pool(name="consts", bufs=1) as consts,
            tc.tile_pool(name="chunk", bufs=3) as chunk,
            tc.tile_pool(name="xtfp", bufs=2) as xtfp,
            tc.tile_pool(name="memp", bufs=2) as memp,
            tc.tile_pool(name="pt", bufs=8) as ptp,
            tc.tile_pool(name="small", bufs=3) as small,
            tc.tile_pool(name="tmpp", bufs=12) as tmpp,
            tc.tile_pool(name="pp", bufs=2, space="PSUM") as pp,
            tc.tile_pool(name="sp", bufs=2, space="PSUM") as spp,
            tc.tile_pool(name="op", bufs=1, space="PSUM") as opp,
            tc.tile_pool(name="tp", bufs=1, space="PSUM") as tpp,
        ):
            # ---- persistent SBUF ----
            wqkv_sb = consts.tile([128, 8, 3 * EC], f8, tag="wqkv")
            wq32_sb = consts.tile([128, 8, EC], f32r, tag="wq32")
            wp_sb = consts.tile([128, 4, E], f16, tag="wp")
            mask_sb = consts.tile([128, 128], f16, tag="mask")
            ident_sb = consts.tile([128, 128], f16, tag="ident")
            kT_sb = consts.tile([128, 4, T], f16, tag="kT")
            v_sb = consts.tile([128, T // 128, HPC, D + 2], f8, tag="v")

            # q/k weight halves first so pass A can start as early as possible
            nc.sync.dma_start(wqkv_sb[:, :, 0 : 2 * EC], wqkv_r[:, :, 0 : 2 * EC])
            nc.sync.dma_start(mask_sb[:], mask)
            nc.sync.dma_start(ident_sb[:], ident)
            # ones column for softmax denominators; 16.0 cancels the WS
            # scaling of v (q,k carry WS each -> exp scale divides by WS^2)
            nc.vector.memset(v_sb[:, :, :, D], WS)

            qT_tiles = [None] * NCHUNK
            combT_tiles = [None] * NCHUNK
            memT_tiles = [None] * NCHUNK

            def emit_passA(c, qT_c, m):
                ts = slice(c * TC, (c + 1) * TC)
                ps = pp.tile([128, TC], f32, tag="pp512")
                for kp in range(4):
                    nc.tensor.matmul(
                        ps[:],
                        wqkv_sb[:, 2 * kp : 2 * kp + 2, 128 * m : 128 * (m + 1)],
                        xtb_tiles[c][:, 2 * kp : 2 * kp + 2, :],
                        start=(kp == 0),
                        stop=(kp == 3),
                        perf_mode=mybir.MatmulPerfMode.DoubleRow,
                    )
                if m < 4:
                    nc.vector.tensor_copy(qT_c[:, m, :], ps[:])
                else:
                    nc.scalar.activation(
                        kT_sb[:, m - 4, ts], ps[:],
                        mybir.ActivationFunctionType.Copy,
                    )

            def emit_passB(c, tb):
                ps = pp.tile([128, TC], f32, tag="pp512")
                for kp in range(4):
                    nc.tensor.matmul(
                        ps[:],
                        xtb_tiles[c][:, 2 * kp : 2 * kp + 2, 128 * tb : 128 * (tb + 1)],
                        wqkv_sb[:, 2 * kp : 2 * kp + 2, 2 * EC : 3 * EC],
                        start=(kp == 0),
                        stop=(kp == 3),
                        perf_mode=mybir.MatmulPerfMode.DoubleRow,
                    )
                nc.vector.tensor_copy(
                    v_sb[:, 4 * c + tb, :, 0:D],
                    ps[:].rearrange("p (h d) -> p h d", d=D),
                )

            def emit_mem_tb(c, qT_c, memT_c, tb):
                trow = slice(c * TC + 128 * tb, c * TC + 128 * (tb + 1))
                # exact fp32 q via float32r matmul (1 cycle/row at 512 wide)
                ps = pp.tile([128, TC], f32, tag="pp512")
                for ke in range(8):
                    nc.tensor.matmul(
                        ps[:],
                        xtf_tiles[c][:, ke, 128 * tb : 128 * (tb + 1)],
                        wq32_sb[:, ke, :],
                        start=(ke == 0),
                        stop=(ke == 7),
                    )
                mk_t = memp.tile([128, KSLOT, EC], f32, tag="mk")
                nc.sync.dma_start(
                    mk_t[:], mk[trow, :].rearrange("p (k e) -> p k e", k=KSLOT)
                )
                mv_t = memp.tile([128, KSLOT, EC], f16, tag="mv")
                nc.sync.dma_start(
                    mv_t[:], mvg[trow, :].rearrange("p (k e) -> p k e", k=KSLOT)
                )

                # scores s3[t, k, h] = sum_d qn*mk, all fp32 (in-place into mk_t)
                prod = mk_t
                nc.vector.tensor_mul(
                    prod[:], mk_t[:], ps[:, None, :].to_broadcast((128, KSLOT, EC))
                )
                s3 = small.tile([128, KSLOT, HPC], f32, tag="s3")
                nc.vector.tensor_reduce(
                    s3[:],
                    prod[:].rearrange("p k (h d) -> p k h d", d=D),
                    mybir.AxisListType.X,
                    mybir.AluOpType.add,
                )
                m3 = small.tile([128, HPC], f32, tag="m3")
                nc.vector.tensor_reduce(
                    m3[:],
                    s3[:].rearrange("p k h -> p h k"),
                    mybir.AxisListType.X,
                    mybir.AluOpType.max,
                )
                z3 = small.tile([128, KSLOT, HPC], f32, tag="z3")
                nc.vector.tensor_sub(
                    z3[:], s3[:], m3[:, None, :].to_broadcast((128, KSLOT, HPC))
                )
                e3 = small.tile([128, KSLOT, HPC], f32, tag="e3")
                nc.scalar.activation(
                    e3[:], z3[:], mybir.ActivationFunctionType.Exp,
                    scale=float(E) * float(np.sqrt(H)),
                )
                den = small.tile([128, HPC], f32, tag="den")
                nc.vector.tensor_reduce(
                    den[:],
                    e3[:].rearrange("p k h -> p h k"),
                    mybir.AxisListType.X,
                    mybir.AluOpType.add,
                )
                rden = small.tile([128, HPC], f32, tag="rden")
                nc.vector.reciprocal(rden[:], den[:])
                w3 = small.tile([128, KSLOT, HPC], f16, tag="w3")
                nc.vector.tensor_mul(
                    w3[:], e3[:], rden[:, None, :].to_broadcast((128, KSLOT, HPC))
                )
                # blend: mm[t, e] = sum_k w3[t,k,h(e)] * mvg[t,k,e]
                wprod = memp.tile([128, KSLOT, EC], f16, tag="wprod")
                for kk in range(KSLOT):
                    nc.gpsimd.tensor_mul(
                        wprod[:, kk, :].rearrange("p (h d) -> p h d", d=D),
                        mv_t[:, kk, :].rearrange("p (h d) -> p h d", d=D),
                        w3[:, kk, :, None].to_broadcast((128, HPC, D)),
                    )
                mm_t = memp.tile([128, EC], f16, tag="mm")
                nc.vector.tensor_add(mm_t[:], wprod[:, 0, :], wprod[:, 1, :])
                nc.vector.tensor_add(mm_t[:], mm_t[:], wprod[:, 2, :])

                for ec in range(4):
                    tps = tpp.tile([128, 128], f16, tag="tp")
                    nc.tensor.transpose(
                        tps[:], mm_t[:, 128 * ec : 128 * (ec + 1)], ident_sb[:]
                    )
                    nc.vector.tensor_copy(
                        memT_c[:, ec, 128 * tb : 128 * (tb + 1)], tps[:]
                    )

            def emit_attn_add(c, combT_c, memT_c, h, tmp):
                prow = slice(64 * (h % 2), 64 * (h % 2) + 64)
                pc = h // 2
                nc.vector.tensor_add(
                    combT_c[prow, pc, :], memT_c[prow, pc, :], tmp[prow, :]
                )

            def emit_attn_head(c, qT_c, combT_c, memT_c, h):
                prow = slice(64 * (h % 2), 64 * (h % 2) + 64)
                pc = h // 2
                ops4 = opp.tile([65, TC], f32, tag="ops")
                # key tiles in pairs: exp of each tile lands in one half of a
                # paired fp8 pt tile; fully-unmasked pairs feed one DoubleRow
                # AV (two key tiles per matmul at 0.5 cycles/row)
                for jp in range(0, 4 * c + 4, 2):
                    pt = ptp.tile([128, 2, TC], f8, tag="pt")
                    sps = spp.tile([128, 2, TC], f32, tag="sps")
                    lo0 = 128 * (jp - 4 * c) if jp >= 4 * c else 0
                    for u in range(2):
                        j = jp + u
                        nc.tensor.matmul(
                            sps[:, u, lo0:TC],
                            kT_sb[prow, pc, 128 * j : 128 * (j + 1)],
                            qT_c[prow, pc, lo0:TC],
                            start=True,
                            stop=True,
                        )
                    # one exp for the pair (over the union of live columns)
                    nc.scalar.activation(
                        pt[:, :, lo0:TC], sps[:, :, lo0:TC],
                        mybir.ActivationFunctionType.Exp,
                        scale=1.0 / (np.sqrt(D) * WS * WS),
                    )
                    if jp + 2 <= 4 * c:
                        nc.tensor.matmul(
                            ops4[:],
                            v_sb[:, jp : jp + 2, h, 0 : D + 1],
                            pt[:],
                            start=(jp == 0),
                            stop=False,
                            skip_group_check=True,
                            perf_mode=mybir.MatmulPerfMode.DoubleRow,
                        )
                    else:
                        for u in range(2):
                            j = jp + u
                            jj = j - 4 * c
                            ptd = ptp.tile([128, 128], f8, tag="ptd")
                            nc.gpsimd.affine_select(
                                out=ptd[:],
                                in_=pt[:, u, 128 * jj : 128 * (jj + 1)],
                                compare_op=mybir.AluOpType.is_ge,
                                fill=0.0,
                                base=0,
                                pattern=[[1, 128]],
                                channel_multiplier=-1,
                            )
                            nc.tensor.matmul(
                                ops4[:, 128 * jj : 128 * (jj + 1)],
                                v_sb[:, j, h, 0 : D + 1],
                                ptd[:],
                                start=(c == 0 and jj == 0),
                                stop=True,
                                skip_group_check=True,
                            )
                            if jj < 3:
                                nc.tensor.matmul(
                                    ops4[:, 128 * (jj + 1) : TC],
                                    v_sb[:, j, h, 0 : D + 1],
                                    pt[:, u, 128 * (jj + 1) : TC],
                                    start=(c == 0 and jj == 0),
                                    stop=False,
                                    skip_group_check=True,
                                )
                # normalize into tmp; the memT+tmp add is emitted separately
                # (after the full memT of this chunk is available)
                rr = small.tile([1, TC], f32, tag="rr")
                nc.vector.reciprocal(rr[:], ops4[64:65, :])
                bc = small.tile([128, TC], f32, tag="bc")
                nc.gpsimd.partition_broadcast(bc[:], rr[0:1, :])
                tmp = tmpp.tile([128, TC], f16, tag="tmpo")
                nc.vector.tensor_mul(tmp[prow, :], ops4[0:64, :], bc[prow, :])
                return tmp

            def emit_cproj(c, combT_c, tb):
                trow = slice(c * TC + 128 * tb, c * TC + 128 * (tb + 1))
                for n in range(2):
                    ps = pp.tile([128, TC], f32, tag="pp512")
                    for ke in range(4):
                        nc.tensor.matmul(
                            ps[:],
                            combT_c[:, ke, 128 * tb : 128 * (tb + 1)],
                            wp_sb[:, ke, TC * n : TC * (n + 1)],
                            start=(ke == 0),
                            stop=(ke == 3),
                        )
                    ost = chunk.tile([128, TC], f32, tag="ost")
                    nc.vector.tensor_copy(ost[:], ps[:])
                    nc.sync.dma_start(out[trow, TC * n : TC * (n + 1)], ost[:])

            # ---- software-pipelined emission ----
            xtb_tiles = {}
            xtf_tiles = {}
            for c in range(NCHUNK + 1):
                if c < NCHUNK:
                    cts = slice(c * TC, (c + 1) * TC)
                    xtb_c = chunk.tile([128, 8, TC], f8, tag="xtb")
                    nc.sync.dma_start(xtb_c[:], xT_r[:, :, cts])
                    xtb_tiles[c] = xtb_c
                    xtf_c = xtfp.tile([128, 8, TC], f32r, tag="xtf")
                    nc.sync.dma_start(xtf_c[:], xTf_r[:, :, cts])
                    xtf_tiles[c] = xtf_c
                    qT_tiles[c] = chunk.tile([128, 4, TC], f16, tag="qT", name=f"qT{c}")
                    combT_tiles[c] = chunk.tile(
                        [128, 4, TC], f16, tag="combT", name=f"combT{c}"
                    )
                    memT_tiles[c] = chunk.tile(
                        [128, 4, TC], f16, tag="memT", name=f"memT{c}"
                    )

                if c == 0:
                    nc.sync.dma_start(
                        wqkv_sb[:, :, 2 * EC : 3 * EC], wqkv_r[:, :, 2 * EC : 3 * EC]
                    )
                    nc.sync.dma_start(wq32_sb[:], wq32_r)
                    nc.sync.dma_start(wp_sb[:], wp_r)
                # fine-grained weave: projections + mem of chunk c between
                # attention heads (late half of c-1, early half of c), so the
                # ACT exp stream is spread evenly and PE/DVE stay fed
                late_tmps = {}
                for k in range(4):
                    if c < NCHUNK:
                        emit_passA(c, qT_tiles[c], 2 * k)
                        emit_passA(c, qT_tiles[c], 2 * k + 1)
                    if k < 2 and c >= 1:
                        emit_mem_tb(c - 1, qT_tiles[c - 1], memT_tiles[c - 1], 2 + k)
                    if c >= 1:
                        late_tmps[4 + k] = emit_attn_head(
                            c - 1, qT_tiles[c - 1], combT_tiles[c - 1],
                            memT_tiles[c - 1], 4 + k
                        )
                    if c < NCHUNK:
                        emit_passB(c, k)
                    if k >= 2 and c < NCHUNK:
                        emit_mem_tb(c, qT_tiles[c], memT_tiles[c], k - 2)
                if c >= 1:
                    # memT(c-1) is complete here: late heads + deferred early
                    for k in range(4):
                        emit_attn_add(
                            c - 1, combT_tiles[c - 1], memT_tiles[c - 1],
                            4 + k, late_tmps[4 + k]
                        )
                    for h in range(4):
                        emit_attn_add(
                            c - 1, combT_tiles[c - 1], memT_tiles[c - 1],
                            h, early_tmps[h]
                        )
                if c < NCHUNK:
                    early_tmps = {}
                    for h in range(4):
                        early_tmps[h] = emit_attn_head(
                            c, qT_tiles[c], combT_tiles[c], memT_tiles[c], h
                        )
                if c >= 1:
                    for tb in range(4):
                        emit_cproj(c - 1, combT_tiles[c - 1], tb)

    nc.compile()
    return nc


def _prep_inputs(x, mem_k, mem_v, W_attn, W_proj, gate_bias):
    """Build per-core input maps (host-side sharding/layout only)."""
    in_maps = []
    g = gate_bias.reshape(H)
    tk = np.arange(128)[:, None]
    tq = np.arange(128)[None, :]
    mask = (tk <= tq).astype(np.float16)
    ident = np.eye(128, dtype=np.float16)
    for core in range(NCORES):
        b, hg = core // 2, core % 2
        cs = slice(hg * EC, (hg + 1) * EC)
        gh = g[hg * HPC : (hg + 1) * HPC].astype(np.float32)   # [8]
        xb = np.asarray(x[b], dtype=np.float32)            # [T, E]
        xT = np.ascontiguousarray(xb.T)                    # [E, T]
        wq = np.ascontiguousarray(W_attn[:, cs])           # [E, 512]
        wk = np.ascontiguousarray(W_attn[:, E + hg * EC : E + (hg + 1) * EC])
        wv = np.ascontiguousarray(W_attn[:, 2 * E + hg * EC : 2 * E + (hg + 1) * EC])
        wv = wv * (1.0 - gh).repeat(D)[None, :]            # fold (1-g) into W_v
        wqkv = np.concatenate([wq, wk, wv], axis=1) * WS   # [E, 1536], fp8 scaling
        mkc = np.ascontiguousarray(mem_k[b][:, :, cs]).reshape(T, KSLOT * EC)
        mvc = np.ascontiguousarray(mem_v[b][:, :, cs]).astype(np.float32)
        # fold gate into mem_v: combined = mem*g + y*(1-g)
        mvc = mvc * gh.repeat(D)[None, None, :]
        mvc = mvc.reshape(T, KSLOT * EC)
        wpc = np.ascontiguousarray(W_proj[cs, :])          # [512, E]
        in_maps.append(
            {
                "xT": xT.astype(ml_dtypes.float8_e4m3),
                "xTf": xT,
                "wqkv": wqkv.astype(ml_dtypes.float8_e4m3),
                "wq32": wq,
                "wp": wpc.astype(np.float16),
                "mk": mkc.astype(np.float32),
                "mvg": mvc.astype(np.float16),
                "mask": mask,
                "ident": ident,
            }
        )
    return in_maps


def kernel(x, mem_k, mem_v, W_attn, W_proj, gate_bias, **kw):
    x = np.asarray(x, dtype=np.float32)
    mem_k = np.asarray(mem_k, dtype=np.float32)
    mem_v = np.asarray(mem_v, dtype=np.float32)
    W_attn = np.asarray(W_attn, dtype=np.float32)
    W_proj = np.asarray(W_proj, dtype=np.float32)
    gate_bias = np.asarray(gate_bias, dtype=np.float32)

    if "nc" not in _CACHE:
        _CACHE["nc"] = _build_nc()
    nc = _CACHE["nc"]
    in_maps = _prep_inputs(x, mem_k, mem_v, W_attn, W_proj, gate_bias)
    res = run_bass_kernel_spmd(nc, in_maps, list(range(NCORES)), **kw)
    results = res.results if hasattr(res, "results") else res
    out = np.empty((B, T, E), dtype=np.float32)
    for b in range(B):
        out[b] = results[2 * b]["out"] + results[2 * b + 1]["out"]
    _CACHE["last_res"] = res
    return out


# revision 4
# speedup vs baseline: 1.0068x; 1.0068x over previous
"""KNN-Attention Trainium2 kernel.

Sharding: 8 cores = 4 batches x 2 head-groups (8 heads each).
Each core computes a partial output [T, E] = combined_slice @ W_proj_slice;
host sums the two partials per batch.

Key design points (vs the straightforward bf16/fp32 version, ~1.8x faster):
  - memory-attention score path stays exact-ish fp32: the softmax scale
    E*sqrt(H)=4096 amplifies q/k errors into slot flips; even fp16 scores
    fail the 2e-2 gate. The q projection uses float32r matmuls, which the
    PE runs at 1 cycle/row for 512-wide tiles (plain fp32 costs 4).
  - qkv projections and the attention AV products run in fp8e4m3 with
    DoubleRow perf mode (two 128-deep contraction tiles per matmul at
    0.5 cycles/row). W_attn is pre-scaled by 16 on the host to avoid fp8
    subnormals; the softmax-denominator ones-column is 16.0 so the scale
    cancels in the normalization. S matmuls / c_proj stay fp16 (precision
    or layout constraints).
  - causal diagonal: 512-wide S/exp tiles, but a triangular gpsimd
    affine_select produces the masked 128x128 diagonal block and the AV
    matmuls skip the dead quadrants.
  - (1-g) folded into W_v columns, g into mem_v on the host.
  - software pipeline across t-chunks: per iteration the emission weaves
    qkv projections of chunk c, the late mem/attention halves of chunk c-1
    and the early halves of chunk c, so PE / ACT (exp stream) / DVE / Pool
    all stay fed; mem results land in a separate memT tile and the
    attention adds (combT = memT + y/den) are deferred until memT is
    complete, keeping program-order dependencies valid.
  - elementwise work spread across DVE / Pool(GPSIMD) / ACT; Pool cannot
    touch PSUM (hardware restriction), so PSUM copies stay on DVE/ACT.
"""

import ml_dtypes
import numpy as np

import concourse.bass as bass
import concourse.mybir as mybir
import concourse.tile as tile
from concourse import bacc
from concourse.bass_utils import run_bass_kernel_spmd

B, T, E, H, KSLOT = 4, 2048, 1024, 16, 3
D = E // H          # 64
HPC = 8             # heads per core
EC = HPC * D        # 512 cols per core
NCORES = 8
TC = 512            # t-chunk
NCHUNK = T // TC    # 4

f32 = mybir.dt.float32
f32r = mybir.dt.float32r
f16 = mybir.dt.float16
f8 = mybir.dt.float8e4
WS = 16.0           # host-side scale on W_attn so fp8 stays out of subnormals

_CACHE = {}


def _build_nc():
    nc = bacc.Bacc("TRN2", target_bir_lowering=False, debug=False)

    # ---- DRAM I/O ----
    xT = nc.dram_tensor("xT", [E, T], f8, kind="ExternalInput").ap()
    xTf = nc.dram_tensor("xTf", [E, T], f32r, kind="ExternalInput").ap()
    wqkv = nc.dram_tensor("wqkv", [E, 3 * EC], f8, kind="ExternalInput").ap()
    wq32 = nc.dram_tensor("wq32", [E, EC], f32r, kind="ExternalInput").ap()
    wp = nc.dram_tensor("wp", [EC, E], f16, kind="ExternalInput").ap()
    mk = nc.dram_tensor("mk", [T, KSLOT * EC], f32, kind="ExternalInput").ap()
    mvg = nc.dram_tensor("mvg", [T, KSLOT * EC], f16, kind="ExternalInput").ap()
    mask = nc.dram_tensor("mask", [128, 128], f16, kind="ExternalInput").ap()
    ident = nc.dram_tensor("ident", [128, 128], f16, kind="ExternalInput").ap()
    out = nc.dram_tensor("out", [T, E], f32, kind="ExternalOutput").ap()

    # partition-tiled DRAM views
    xT_r = xT.rearrange("(ko p) t -> p ko t", p=128)        # [128, 8, T]
    xTf_r = xTf.rearrange("(ko p) t -> p ko t", p=128)      # [128, 8, T]
    wqkv_r = wqkv.rearrange("(ko p) n -> p ko n", p=128)    # [128, 8, 1536]
    wq32_r = wq32.rearrange("(ko p) n -> p ko n", p=128)    # [128, 8, 512]
    wp_r = wp.rearrange("(ko p) n -> p ko n", p=128)        # [128, 4, 1024]

    with tile.TileContext(nc) as tc:
        with (
            tc.tile_pool(name="consts", bufs=1) as consts,
            tc.tile_pool(name="chunk", bufs=3) as chunk,
            tc.tile_pool(name="xtfp", bufs=2) as xtfp,
            tc.tile_pool(name="memp", bufs=2) as memp,
            tc.tile_pool(name="pt", bufs=8) as ptp,
            tc.tile_pool(name="small", bufs=3) as small,
            tc.tile_pool(name="tmpp", bufs=12) as tmpp,
            tc.tile_pool(name="pp", bufs=2, space="PSUM") as pp,
            tc.tile_pool(name="sp", bufs=2, space="PSUM") as spp,
            tc.tile_pool(name="op", bufs=1, space="PSUM") as opp,
            tc.tile_pool(name="tp", bufs=1, space="PSUM") as tpp,
        ):
            # ---- persistent SBUF ----
            wqkv_sb = consts.tile([128, 8, 3 * EC], f8, tag="wqkv")
            wq32_sb = consts.tile([128, 8, EC], f32r, tag="wq32")
            wp_sb = consts.tile([128, 4, E], f16, tag="wp")
            mask_sb = consts.tile([128, 128], f16, tag="mask")
            ident_sb = consts.tile([128, 128], f16, tag="ident")
            kT_sb = consts.tile([128, 4, T], f16, tag="kT")
            v_sb = consts.tile([128, T // 128, HPC, D + 2], f8, tag="v")

            # q/k weight halves first so pass A can start as early as possible
            nc.sync.dma_start(wqkv_sb[:, :, 0 : 2 * EC], wqkv_r[:, :, 0 : 2 * EC])
            nc.sync.dma_start(mask_sb[:], mask)
            nc.sync.dma_start(ident_sb[:], ident)
            # ones column for softmax denominators; 16.0 cancels the WS
            # scaling of v (q,k carry WS each -> exp scale divides by WS^2)
            nc.vector.memset(v_sb[:, :, :, D], WS)

            qT_tiles = [None] * NCHUNK
            combT_tiles = [None] * NCHUNK
            memT_tiles = [None] * NCHUNK

            def emit_passA(c, qT_c, m):
                ts = slice(c * TC, (c + 1) * TC)
                ps = pp.tile([128, TC], f32, tag="pp512")
                for kp in range(4):
                    nc.tensor.matmul(
                        ps[:],
                        wqkv_sb[:, 2 * kp : 2 * kp + 2, 128 * m : 128 * (m + 1)],
                        xtb_tiles[c][:, 2 * kp : 2 * kp + 2, :],
                        start=(kp == 0),
                        stop=(kp == 3),
                        perf_mode=mybir.MatmulPerfMode.DoubleRow,
                    )
                if m < 4:
                    nc.vector.tensor_copy(qT_c[:, m, :], ps[:])
                else:
                    nc.scalar.activation(
                        kT_sb[:, m - 4, ts], ps[:],
                        mybir.ActivationFunctionType.Copy,
                    )

            def emit_passB(c, tb):
                ps = pp.tile([128, TC], f32, tag="pp512")
                for kp in range(4):
                    nc.tensor.matmul(
                        ps[:],
                        xtb_tiles[c][:, 2 * kp : 2 * kp + 2, 128 * tb : 128 * (tb + 1)],
                        wqkv_sb[:, 2 * kp : 2 * kp + 2, 2 * EC : 3 * EC],
                        start=(kp == 0),
                        stop=(kp == 3),
                        perf_mode=mybir.MatmulPerfMode.DoubleRow,
                    )
                nc.vector.tensor_copy(
                    v_sb[:, 4 * c + tb, :, 0:D],
                    ps[:].rearrange("p (h d) -> p h d", d=D),
                )

            def emit_mem_tb(c, qT_c, memT_c, tb):
                trow = slice(c * TC + 128 * tb, c * TC + 128 * (tb + 1))
                # exact fp32 q via float32r matmul (1 cycle/row at 512 wide)
                ps = pp.tile([128, TC], f32, tag="pp512")
                for ke in range(8):
                    nc.tensor.matmul(
                        ps[:],
                        xtf_tiles[c][:, ke, 128 * tb : 128 * (tb + 1)],
                        wq32_sb[:, ke, :],
                        start=(ke == 0),
                        stop=(ke == 7),
                    )
                mk_t = memp.tile([128, KSLOT, EC], f32, tag="mk")
                nc.sync.dma_start(
                    mk_t[:], mk[trow, :].rearrange("p (k e) -> p k e", k=KSLOT)
                )
                mv_t = memp.tile([128, KSLOT, EC], f16, tag="mv")
                nc.sync.dma_start(
                    mv_t[:], mvg[trow, :].rearrange("p (k e) -> p k e", k=KSLOT)
                )

                # scores s3[t, k, h] = sum_d qn*mk, all fp32 (in-place into mk_t)
                prod = mk_t
                nc.vector.tensor_mul(
                    prod[:], mk_t[:], ps[:, None, :].to_broadcast((128, KSLOT, EC))
                )
                s3 = small.tile([128, KSLOT, HPC], f32, tag="s3")
                nc.vector.tensor_reduce(
                    s3[:],
                    prod[:].rearrange("p k (h d) -> p k h d", d=D),
                    mybir.AxisListType.X,
                    mybir.AluOpType.add,
                )
                m3 = small.tile([128, HPC], f32, tag="m3")
                nc.vector.tensor_reduce(
                    m3[:],
                    s3[:].rearrange("p k h -> p h k"),
                    mybir.AxisListType.X,
                    mybir.AluOpType.max,
                )
                z3 = small.tile([128, KSLOT, HPC], f32, tag="z3")
                nc.vector.tensor_sub(
                    z3[:], s3[:], m3[:, None, :].to_broadcast((128, KSLOT, HPC))
                )
                e3 = small.tile([128, KSLOT, HPC], f32, tag="e3")
                nc.scalar.activation(
                    e3[:], z3[:], mybir.ActivationFunctionType.Exp,
                    scale=float(E) * float(np.sqrt(H)),
                )
                den = small.tile([128, HPC], f32, tag="den")
                nc.vector.tensor_reduce(
                    den[:],
                    e3[:].rearrange("p k h -> p h k"),
                    mybir.AxisListType.X,
                    mybir.AluOpType.add,
                )
                rden = small.tile([128, HPC], f32, tag="rden")
                nc.vector.reciprocal(rden[:], den[:])
                w3 = small.tile([128, KSLOT, HPC], f16, tag="w3")
                nc.vector.tensor_mul(
                    w3[:], e3[:], rden[:, None, :].to_broadcast((128, KSLOT, HPC))
                )
                # blend: mm[t, e] = sum_k w3[t,k,h(e)] * mvg[t,k,e]
                wprod = memp.tile([128, KSLOT, EC], f16, tag="wprod")
                for kk in range(KSLOT):
                    nc.gpsimd.tensor_mul(
                        wprod[:, kk, :].rearrange("p (h d) -> p h d", d=D),
                        mv_t[:, kk, :].rearrange("p (h d) -> p h d", d=D),
                        w3[:, kk, :, None].to_broadcast((128, HPC, D)),
                    )
                mm_t = memp.tile([128, EC], f16, tag="mm")
                nc.vector.tensor_add(mm_t[:], wprod[:, 0, :], wprod[:, 1, :])
                nc.vector.tensor_add(mm_t[:], mm_t[:], wprod[:, 2, :])

                for ec in range(4):
                    tps = tpp.tile([128, 128], f16, tag="tp")
                    nc.tensor.transpose(
                        tps[:], mm_t[:, 128 * ec : 128 * (ec + 1)], ident_sb[:]
                    )
                    nc.vector.tensor_copy(
                        memT_c[:, ec, 128 * tb : 128 * (tb + 1)], tps[:]
                    )

            def emit_attn_add(c, combT_c, memT_c, h, tmp):
                prow = slice(64 * (h % 2), 64 * (h % 2) + 64)
                pc = h // 2
                nc.vector.tensor_add(
                    combT_c[prow, pc, :], memT_c[prow, pc, :], tmp[prow, :]
                )

            def emit_attn_head(c, qT_c, combT_c, memT_c, h):
                prow = slice(64 * (h % 2), 64 * (h % 2) + 64)
                pc = h // 2
                ops4 = opp.tile([65, TC], f32, tag="ops")
                # key tiles in pairs: exp of each tile lands in one half of a
                # paired fp8 pt tile; fully-unmasked pairs feed one DoubleRow
                # AV (two key tiles per matmul at 0.5 cycles/row)
                for jp in range(0, 4 * c + 4, 2):
                    pt = ptp.tile([128, 2, TC], f8, tag="pt")
                    sps = spp.tile([128, 2, TC], f32, tag="sps")
                    lo0 = 128 * (jp - 4 * c) if jp >= 4 * c else 0
                    for u in range(2):
                        j = jp + u
                        nc.tensor.matmul(
                            sps[:, u, lo0:TC],
                            kT_sb[prow, pc, 128 * j : 128 * (j + 1)],
                            qT_c[prow, pc, lo0:TC],
                            start=True,
                            stop=True,
                        )
                    # one exp for the pair (over the union of live columns)
                    nc.scalar.activation(
                        pt[:, :, lo0:TC], sps[:, :, lo0:TC],
                        mybir.ActivationFunctionType.Exp,
                        scale=1.0 / (np.sqrt(D) * WS * WS),
                    )
                    if jp + 2 <= 4 * c:
                        nc.tensor.matmul(
                            ops4[:],
                            v_sb[:, jp : jp + 2, h, 0 : D + 1],
                            pt[:],
                            start=(jp == 0),
                            stop=False,
                            skip_group_check=True,
                            perf_mode=mybir.MatmulPerfMode.DoubleRow,
                        )
                    else:
                        for u in range(2):
                            j = jp + u
                            jj = j - 4 * c
                            ptd = ptp.tile([128, 128], f8, tag="ptd")
                            nc.gpsimd.affine_select(
                                out=ptd[:],
                                in_=pt[:, u, 128 * jj : 128 * (jj + 1)],
                                compare_op=mybir.AluOpType.is_ge,
                                fill=0.0,
                                base=0,
                                pattern=[[1, 128]],
                                channel_multiplier=-1,
                            )
                            nc.tensor.matmul(
                                ops4[:, 128 * jj : 128 * (jj + 1)],
                                v_sb[:, j, h, 0 : D + 1],
                                ptd[:],
                                start=(c == 0 and jj == 0),
                                stop=True,
                                skip_group_check=True,
                            )
                            if jj < 3:
                                nc.tensor.matmul(
                                    ops4[:, 128 * (jj + 1) : TC],
                                    v_sb[:, j, h, 0 : D + 1],
                                    pt[:, u, 128 * (jj + 1) : TC],
                                    start=(c == 0 and jj == 0),
                                    stop=False,
                                    skip_group_check=True,
                                )
                # normalize into tmp; the memT+tmp add is emitted separately
                # (after the full memT of this chunk is available)
                rr = small.tile([1, TC], f32, tag="rr")
                nc.vector.reciprocal(rr[:], ops4[64:65, :])
                bc = small.tile([128, TC], f32, tag="bc")
                nc.gpsimd.partition_broadcast(bc[:], rr[0:1, :])
                tmp = tmpp.tile([128, TC], f16, tag="tmpo")
                nc.vector.tensor_mul(tmp[prow, :], ops4[0:64, :], bc[prow, :])
                return tmp

            def emit_cproj(c, combT_c, tb):
                trow = slice(c * TC + 128 * tb, c * TC + 128 * (tb + 1))
                for n in range(2):
                    ps = pp.tile([128, TC], f32, tag="pp512")
                    for ke in range(4):
                        nc.tensor.matmul(
                            ps[:],
                            combT_c[:, ke, 128 * tb : 128 * (tb + 1)],
                            wp_sb[:, ke, TC * n : TC * (n + 1)],
                            start=(ke == 0),
                            stop=(ke == 3),
                        )
                    ost = chunk.tile([128, TC], f32, tag="ost")
                    nc.vector.tensor_copy(ost[:], ps[:])
                    nc.sync.dma_start(out[trow, TC * n : TC * (n + 1)], ost[:])

            # ---- software-pipelined emission ----
            xtb_tiles = {}
            xtf_tiles = {}
            for c in range(NCHUNK + 1):
                if c < NCHUNK:
                    cts = slice(c * TC, (c + 1) * TC)
                    xtb_c = chunk.tile([128, 8, TC], f8, tag="xtb")
                    nc.sync.dma_start(xtb_c[:], xT_r[:, :, cts])
                    xtb_tiles[c] = xtb_c
                    xtf_c = xtfp.tile([128, 8, TC], f32r, tag="xtf")
                    nc.sync.dma_start(xtf_c[:], xTf_r[:, :, cts])
                    xtf_tiles[c] = xtf_c
                    qT_tiles[c] = chunk.tile([128, 4, TC], f16, tag="qT", name=f"qT{c}")
                    combT_tiles[c] = chunk.tile(
                        [128, 4, TC], f16, tag="combT", name=f"combT{c}"
                    )
                    memT_tiles[c] = chunk.tile(
                        [128, 4, TC], f16, tag="memT", name=f"memT{c}"
                    )

                if c == 0:
                    nc.sync.dma_start(
                        wqkv_sb[:, :, 2 * EC : 3 * EC], wqkv_r[:, :, 2 * EC : 3 * EC]
                    )
                    nc.sync.dma_start(wq32_sb[:], wq32_r)
                    nc.sync.dma_start(wp_sb[:], wp_r)
                # fine-grained weave: projections + mem of chunk c between
                # attention heads (late half of c-1, early half of c), so the
                # ACT exp stream is spread evenly and PE/DVE stay fed
                late_tmps = {}
                for k in range(4):
                    if c < NCHUNK:
                        emit_passA(c, qT_tiles[c], 2 * k)
                        emit_passA(c, qT_tiles[c], 2 * k + 1)
                    if k < 2 and c >= 1:
                        emit_mem_tb(c - 1, qT_tiles[c - 1], memT_tiles[c - 1], 2 + k)
                    if c >= 1:
                        late_tmps[4 + k] = emit_attn_head(
                            c - 1, qT_tiles[c - 1], combT_tiles[c - 1],
                            memT_tiles[c - 1], 4 + k
                        )
                    if c < NCHUNK:
                        emit_passB(c, k)
                    if k >= 2 and c < NCHUNK:
                        emit_mem_tb(c, qT_tiles[c], memT_tiles[c], k - 2)
                if c >= 1:
                    # memT(c-1) is complete here: late heads + deferred early
                    for k in range(4):
                        emit_attn_add(
                            c - 1, combT_tiles[c - 1], memT_tiles[c - 1],
                            4 + k, late_tmps[4 + k]
                        )
                    for h in range(4):
                        emit_attn_add(
                            c - 1, combT_tiles[c - 1], memT_tiles[c - 1],
                            h, early_tmps[h]
                        )
                if c < NCHUNK:
                    early_tmps = {}
                    for h in range(4):
                        early_tmps[h] = emit_attn_head(
                            c, qT_tiles[c], combT_tiles[c], memT_tiles[c], h
                        )
                if c >= 1:
                    for tb in range(4):
                        emit_cproj(c - 1, combT_tiles[c - 1], tb)

    nc.compile()
    return nc


def _prep_inputs(x, mem_k, mem_v, W_attn, W_proj, gate_bias):
    """Build per-core input maps (host-side sharding/layout only)."""
    in_maps = []
    g = gate_bias.reshape(H)
    tk = np.arange(128)[:, None]
    tq = np.arange(128)[None, :]
    mask = (tk <= tq).astype(np.float16)
    ident = np.eye(128, dtype=np.float16)
    for core in range(NCORES):
        b, hg = core // 2, core % 2
        cs = slice(hg * EC, (hg + 1) * EC)
        gh = g[hg * HPC : (hg + 1) * HPC].astype(np.float32)   # [8]
        xb = np.asarray(x[b], dtype=np.float32)            # [T, E]
        xT = np.ascontiguousarray(xb.T)                    # [E, T]
        wq = np.ascontiguousarray(W_attn[:, cs])           # [E, 512]
        wk = np.ascontiguousarray(W_attn[:, E + hg * EC : E + (hg + 1) * EC])
        wv = np.ascontiguousarray(W_attn[:, 2 * E + hg * EC : 2 * E + (hg + 1) * EC])
        wv = wv * (1.0 - gh).repeat(D)[None, :]            # fold (1-g) into W_v
        wqkv = np.concatenate([wq, wk, wv], axis=1) * WS   # [E, 1536], fp8 scaling
        mkc = np.ascontiguousarray(mem_k[b][:, :, cs]).reshape(T, KSLOT * EC)
        mvc = np.ascontiguousarray(mem_v[b][:, :, cs]).astype(np.float32)
        # fold gate into mem_v: combined = mem*g + y*(1-g)
        mvc = mvc * gh.repeat(D)[None, None, :]
        mvc = mvc.reshape(T, KSLOT * EC)
        wpc = np.ascontiguousarray(W_proj[cs, :])          # [512, E]
        in_maps.append(
            {
                "xT": xT.astype(ml_dtypes.float8_e4m3),
                "xTf": xT,
                "wqkv": wqkv.astype(ml_dtypes.float8_e4m3),
                "wq32": wq,
                "wp": wpc.astype(np.float16),
                "mk": mkc.astype(np.float32),
                "mvg": mvc.astype(np.float16),
                "mask": mask,
                "ident": ident,
            }
        )
    return in_maps


def kernel(x, mem_k, mem_v, W_attn, W_proj, gate_bias, **kw):
    x = np.asarray(x, dtype=np.float32)
    mem_k = np.asarray(mem_k, dtype=np.float32)
    mem_v = np.asarray(mem_v, dtype=np.float32)
    W_attn = np.asarray(W_attn, dtype=np.float32)
    W_proj = np.asarray(W_proj, dtype=np.float32)
    gate_bias = np.asarray(gate_bias, dtype=np.float32)

    if "nc" not in _CACHE:
        _CACHE["nc"] = _build_nc()
    nc = _CACHE["nc"]
    in_maps = _prep_inputs(x, mem_k, mem_v, W_attn, W_proj, gate_bias)
    res = run_bass_kernel_spmd(nc, in_maps, list(range(NCORES)), **kw)
    results = res.results if hasattr(res, "results") else res
    out = np.empty((B, T, E), dtype=np.float32)
    for b in range(B):
        out[b] = results[2 * b]["out"] + results[2 * b + 1]["out"]
    _CACHE["last_res"] = res
    return out
